# revision 14
# baseline (speedup 1.0000x reference)
"""Trainium2 Bass kernel for nn_BasicBlock_HMU (two HMU layers + sync BN + residual).

Sharding: data-parallel over batch (8 cores x 512 rows); params replicated.
BN batch statistics are all-gathered across the 8 cores (sync BN).

v3.1 — n-on-partitions orientation, fp8e4m3 DoubleRow with full error
compensation (validated in numpy emulation at rel_err ~6.4e-3):
  quad^T [n_p, b_f] = lam_n*(|x|^2 - 2 mu.x + |mu|^2) + sum_k (v_k.x - v_k.mu)^2
  - DoubleRow fp8 matmuls contract 256 rows at 0.5 cyc/row.
  - precision recovery (per 128-wide n-tile j, per k):
      proj*16 ~ v8.m8 + vr8.(m8/32) + v832.(mr32)
      with v8 = fp8(16v), vr8 = fp8((16v - v8)*32), v832 = fp8(v8/32),
      m8 = fp8(moving), mr32 = fp8((moving - m8)*32), m8/32 exact shift.
    Layer-1 moving is x - 0.5 (halves fp8 quantization error; shift
    constants folded exactly into the f32 bias columns with exact weights).
  - lam applied per-partition in the DVE combine (q*lam + sum sq_k), so
    mu-part quantization is lam-damped; |x|^2 / |h|^2 rows split fp8 hi/lo.
  - BN stats free via Act/DVE accum_out (S1 = sum e, A = sum(e^2-e)).
  - layer-1 output feeds layer-2 directly (no transposes); output written
    n-major, transposed on host. W1 and W2 both streamed just-in-time.
"""

import numpy as np
import ml_dtypes

import concourse.bacc as bacc
import concourse.mybir as mybir
import concourse.tile as tile

try:
    from concourse.bass_utils import run_bass_kernel_spmd
except ImportError:  # pragma: no cover
    from bass_utils import run_bass_kernel_spmd

F32 = mybir.dt.float32
BF16 = mybir.dt.bfloat16
FP8 = mybir.dt.float8e4
Alu = mybir.AluOpType
Act = mybir.ActivationFunctionType
DR = mybir.MatmulPerfMode.DoubleRow
BF = ml_dtypes.bfloat16
F8 = mybir.dt.np(mybir.dt.float8e4)

N_CORES = 8
B, D, N, K = 4096, 1024, 1024, 4
BS = B // N_CORES          # 512 rows per core
NT = N // 128              # 8 n-tiles per layer
CH = D // 128              # 8 contraction chunks (4 DoubleRow pairs)
NBLK = 1 + 3 * K           # mu + (v8, vr8, v832) x4 blocks per n-tile group
GW = NBLK * 1024           # packed group width (13312)
WCOL = NT * GW             # packed weight columns per layer
BN_EPS = 1e-5
C1 = 1024.0 / 3.0          # host-side shift of the |x|^2 row

_CACHE = {}


def _build_nc(reps=1, loop_reps=0, collectives=True):
    nc = bacc.Bacc("TRN2", target_bir_lowering=False, debug=False,
                   num_devices=N_CORES)

    x8_s = nc.dram_tensor("x8_s", [128, CH * BS], FP8, kind="ExternalInput").ap()
    xlo_s = nc.dram_tensor("xlo_s", [128, CH * BS], FP8, kind="ExternalInput").ap()
    xr32_s = nc.dram_tensor("xr32_s", [128, CH * BS], FP8, kind="ExternalInput").ap()
    xres_s = nc.dram_tensor("xres_s", [128, CH * BS], BF16, kind="ExternalInput").ap()
    sqr_s = nc.dram_tensor("sqr_s", [1, 2 * BS], FP8, kind="ExternalInput").ap()
    W1p = nc.dram_tensor("W1p", [128, WCOL], FP8, kind="ExternalInput").ap()
    W2p = nc.dram_tensor("W2p", [128, WCOL], FP8, kind="ExternalInput").ap()
    lamc_s = nc.dram_tensor("lamc_s", [128, 16], F32, kind="ExternalInput").ap()
    cexp_s = nc.dram_tensor("cexp_s", [128, 16], F32, kind="ExternalInput").ap()
    cv_s = nc.dram_tensor("cv_s", [128, 64], F32, kind="ExternalInput").ap()
    gb_s = nc.dram_tensor("gb_s", [128, 32], F32, kind="ExternalInput").ap()
    k1_s = nc.dram_tensor("k1_s", [1, 256], FP8, kind="ExternalInput").ap()
    ones_s = nc.dram_tensor("ones_s", [128, 1], FP8, kind="ExternalInput").ap()
    outT = nc.dram_tensor("outT", [N, BS], F32, kind="ExternalOutput").ap()

    def dr(ap):
        return ap.rearrange("p (two m) -> p two m", two=2)

    with tile.TileContext(nc) as tc:
        with (
            tc.tile_pool(name="const", bufs=1) as constp,
            tc.tile_pool(name="big", bufs=1) as bigp,
            tc.tile_pool(name="wp", bufs=2) as wp,
            tc.tile_pool(name="scr", bufs=2) as scr,
            tc.tile_pool(name="rowp", bufs=1) as rowp,
            tc.tile_pool(name="fin", bufs=2) as finp,
            tc.tile_pool(name="pq", bufs=2, space="PSUM") as pq,
            tc.tile_pool(name="pp", bufs=4, space="PSUM") as pp,
            tc.tile_pool(name="ph", bufs=1, space="PSUM") as php,
            tc.tile_pool(name="dram", bufs=2, space="DRAM") as dramp,
        ):
            # ---- constants (loaded once, shared across reps) ----
            sqr = constp.tile([1, 2 * BS], FP8)
            nc.scalar.dma_start(sqr[:], sqr_s)
            k1r = constp.tile([1, 256], FP8)
            nc.scalar.dma_start(k1r[:], k1_s)
            lamc = constp.tile([128, 16], F32)
            nc.scalar.dma_start(lamc[:], lamc_s)
            cexp = constp.tile([128, 16], F32)
            nc.scalar.dma_start(cexp[:], cexp_s)
            cv = constp.tile([128, 64], F32)
            nc.scalar.dma_start(cv[:], cv_s)
            gb = constp.tile([128, 32], F32)
            nc.scalar.dma_start(gb[:], gb_s)
            onesc = constp.tile([128, 1], FP8)
            nc.scalar.dma_start(onesc[:], ones_s)
            c1eps = constp.tile([128, 1], F32)
            nc.gpsimd.memset(c1eps[:], 1.0 + BN_EPS)
            warm = constp.tile([1, 1], F32)

            def body():
                x8 = bigp.tile([128, CH * BS], FP8, tag="x8")
                nc.scalar.dma_start(x8[:], x8_s)
                xlo = bigp.tile([128, CH * BS], FP8, tag="xlo")
                nc.scalar.dma_start(xlo[:], xlo_s)
                xr32 = bigp.tile([128, CH * BS], FP8, tag="xr32")
                nc.scalar.dma_start(xr32[:], xr32_s)
                xres = bigp.tile([128, CH * BS], BF16, tag="xres")
                h8 = bigp.tile([128, NT * BS], FP8, tag="h8")
                h832 = bigp.tile([128, NT * BS], FP8, tag="h832")
                hr32 = bigp.tile([128, NT * BS], FP8, tag="hr32")
                hsqrow = rowp.tile([1, 2 * BS], FP8, tag="hsqrow")

                # PE p-state warm-up while the first DMAs land
                trash = php.tile([1, BS], F32, tag="trash")
                for _ in range(12):
                    nc.tensor.matmul(trash[:], k1r[0:1, 0:1],
                                     sqr[0:1, 0:BS], start=True, stop=True)

                for L in range(2):
                    m8, mlo, mr32 = ((x8, xlo, xr32), (h8, h832, hr32))[L]
                    srow = (sqr, hsqrow)[L]
                    Wp = (W1p, W2p)[L]
                    e_all = bigp.tile([128, NT * BS], F32, tag="e")
                    stats = rowp.tile([128, 16], F32, tag="stats")

                    # ---- sweep: per n-tile, mu part then 4 v parts ----
                    for j in range(NT):
                        wt = wp.tile([128, GW], FP8, tag="w")
                        nc.sync.dma_start(wt[:], Wp[:, j * GW:(j + 1) * GW])
                        q = pq.tile([128, BS], F32, tag="q")
                        for c in range(CH // 2):
                            nc.tensor.matmul(
                                q[:], dr(wt[:, c * 256:(c + 1) * 256]),
                                dr(m8[:, c * 2 * BS:(c + 1) * 2 * BS]),
                                start=(c == 0), stop=False, perf_mode=DR)
                        nc.tensor.matmul(q[:], k1r[0:1, 0:128], srow[0:1, 0:BS],
                                         start=False, stop=False)
                        nc.tensor.matmul(q[:], k1r[0:1, 128:256],
                                         srow[0:1, BS:2 * BS],
                                         start=False, stop=True)
                        sqk = []
                        for k in range(K):
                            w8 = (1 + 3 * k) * 1024
                            wlo = w8 + 1024
                            w32 = w8 + 2048
                            p = pp.tile([128, BS], F32, tag="p")
                            for c in range(CH // 2):
                                nc.tensor.matmul(
                                    p[:], dr(wt[:, w8 + c * 256: w8 + (c + 1) * 256]),
                                    dr(m8[:, c * 2 * BS:(c + 1) * 2 * BS]),
                                    start=(c == 0), stop=False, perf_mode=DR)
                            for c in range(CH // 2):
                                nc.tensor.matmul(
                                    p[:], dr(wt[:, wlo + c * 256: wlo + (c + 1) * 256]),
                                    dr(mlo[:, c * 2 * BS:(c + 1) * 2 * BS]),
                                    start=False, stop=False, perf_mode=DR)
                            for c in range(CH // 2):
                                nc.tensor.matmul(
                                    p[:], dr(wt[:, w32 + c * 256: w32 + (c + 1) * 256]),
                                    dr(mr32[:, c * 2 * BS:(c + 1) * 2 * BS]),
                                    start=False, stop=(c == CH // 2 - 1),
                                    perf_mode=DR)
                            sk = scr.tile([128, BS], F32, tag="sq", bufs=6)
                            ci = L * 32 + j * 4 + k
                            nc.scalar.activation(sk[:], p[:], Act.Square,
                                                 scale=1.0 / 16.0,
                                                 bias=cv[:, ci:ci + 1])
                            sqk.append(sk)
                        s01 = scr.tile([128, BS], F32, tag="s01")
                        nc.gpsimd.tensor_tensor(out=s01[:], in0=sqk[0][:],
                                                in1=sqk[1][:], op=Alu.add)
                        s23 = scr.tile([128, BS], F32, tag="s23")
                        nc.vector.tensor_tensor(out=s23[:], in0=sqk[2][:],
                                                in1=sqk[3][:], op=Alu.add)
                        s03 = scr.tile([128, BS], F32, tag="s03")
                        nc.gpsimd.tensor_tensor(out=s03[:], in0=s01[:],
                                                in1=s23[:], op=Alu.add)
                        qf = scr.tile([128, BS], F32, tag="qf")
                        nc.vector.scalar_tensor_tensor(
                            out=qf[:], in0=q[:],
                            scalar=lamc[:, L * 8 + j: L * 8 + j + 1],
                            in1=s03[:], op0=Alu.mult, op1=Alu.add)
                        ej = e_all[:, j * BS:(j + 1) * BS]
                        nc.scalar.activation(ej, qf[:], Act.Exp,
                                             scale=-1.0 / D,
                                             bias=cexp[:, L * 8 + j: L * 8 + j + 1],
                                             accum_out=stats[:, j:j + 1])
                        # A_j = sum(e^2 - e); S2 = A - S1 + B recovered later
                        scrq = scr.tile([128, BS], F32, tag="scrq")
                        nc.vector.scalar_tensor_tensor(
                            out=scrq[:], in0=ej, scalar=-1.0, in1=ej,
                            op0=Alu.add, op1=Alu.mult,
                            accum_out=stats[:, 8 + j:9 + j])

                    if L == 0:
                        # residual x needed only for the layer-2 epilogue
                        nc.gpsimd.dma_start(xres[:], xres_s)

                    # ---- sync BN: stats -> AllGather -> reduce ----
                    cin = dramp.tile([128, 16], F32, tag="cin")
                    nc.scalar.dma_start(cin[:], stats[:])
                    cout = dramp.tile([N_CORES * 128, 16], F32, tag="cout",
                                      addr_space="Shared")
                    if collectives:
                        nc.gpsimd.collective_compute(
                            "AllGather", Alu.bypass,
                            replica_groups=[list(range(N_CORES))],
                            ins=[cin[:].opt()], outs=[cout[:].opt()])
                    else:
                        nc.sync.dma_start(cout[0:128, :], cin[:])
                    # preload the ACT Sqrt table off the critical path
                    nc.scalar.activation(warm[:], c1eps[0:1, 0:1], Act.Sqrt)
                    gath = rowp.tile([128, N_CORES * 16], F32, tag="gath")
                    for g in range(N_CORES):
                        nc.scalar.dma_start(
                            gath[:, g * 16:(g + 1) * 16],
                            cout[g * 128:(g + 1) * 128, :])
                    red = rowp.tile([128, 16], F32, tag="red")
                    nc.vector.tensor_reduce(
                        out=red[:],
                        in_=gath[:].rearrange("p (g f) -> p f g", g=N_CORES),
                        axis=mybir.AxisListType.X, op=Alu.add)

                    # ---- finalize: s = g*rsqrt(var+eps), u = b - s*mean ----
                    m_e = finp.tile([128, 8], F32, tag="m_e")
                    nc.vector.tensor_scalar(out=m_e[:], in0=red[:, 0:8],
                                            scalar1=1.0 / B, scalar2=None,
                                            op0=Alu.mult)
                    mz = finp.tile([128, 8], F32, tag="mz")
                    nc.vector.tensor_scalar(out=mz[:], in0=m_e[:],
                                            scalar1=-1.0, scalar2=None,
                                            op0=Alu.add)
                    mz2 = finp.tile([128, 8], F32, tag="mz2")
                    nc.vector.tensor_tensor(out=mz2[:], in0=mz[:], in1=mz[:],
                                            op=Alu.mult)
                    ams = finp.tile([128, 8], F32, tag="ams")
                    nc.vector.tensor_tensor(out=ams[:], in0=red[:, 8:16],
                                            in1=red[:, 0:8], op=Alu.subtract)
                    varr = finp.tile([128, 8], F32, tag="varr")
                    nc.vector.scalar_tensor_tensor(
                        out=varr[:], in0=ams[:], scalar=1.0 / B,
                        in1=mz2[:], op0=Alu.mult, op1=Alu.subtract)
                    sd = finp.tile([128, 8], F32, tag="sd")
                    nc.scalar.activation(sd[:], varr[:], Act.Sqrt, bias=c1eps[:])
                    rs = finp.tile([128, 8], F32, tag="rs")
                    nc.vector.reciprocal(rs[:], sd[:])
                    s_t = finp.tile([128, 8], F32, tag="s_t")
                    nc.vector.tensor_tensor(out=s_t[:], in0=rs[:],
                                            in1=gb[:, 16 * L:16 * L + 8],
                                            op=Alu.mult)
                    um = finp.tile([128, 8], F32, tag="um")
                    nc.vector.tensor_tensor(out=um[:], in0=s_t[:], in1=m_e[:],
                                            op=Alu.mult)
                    u_t = finp.tile([128, 8], F32, tag="u_t")
                    nc.vector.tensor_tensor(out=u_t[:],
                                            in0=gb[:, 16 * L + 8:16 * L + 16],
                                            in1=um[:], op=Alu.subtract)

                    # ---- normalize (+ |h|^2 rows | + residual & store) ----
                    if L == 0:
                        hsqp = php.tile([1, BS], F32, tag="hsq")
                        for j in range(NT):
                            js = slice(j * BS, (j + 1) * BS)
                            hf = scr.tile([128, BS], F32, tag="hf", bufs=3)
                            nc.vector.tensor_scalar(
                                out=hf[:], in0=e_all[:, js],
                                scalar1=s_t[:, j:j + 1], scalar2=u_t[:, j:j + 1],
                                op0=Alu.mult, op1=Alu.add)
                            nc.gpsimd.tensor_copy(h8[:, js], hf[:])
                            nc.gpsimd.tensor_scalar(
                                out=h832[:, js], in0=h8[:, js],
                                scalar1=1.0 / 32.0, scalar2=None, op0=Alu.mult)
                            hd8 = scr.tile([128, BS], F32, tag="hd8")
                            nc.vector.tensor_tensor(out=hd8[:], in0=hf[:],
                                                    in1=h8[:, js],
                                                    op=Alu.subtract)
                            nc.vector.tensor_scalar(
                                out=hr32[:, js], in0=hd8[:],
                                scalar1=32.0, scalar2=None, op0=Alu.mult)
                            hh = scr.tile([128, BS], FP8, tag="hh")
                            nc.scalar.activation(
                                hh[:], e_all[:, js], Act.Square,
                                scale=s_t[:, j:j + 1], bias=u_t[:, j:j + 1])
                            nc.tensor.matmul(hsqp[:], onesc[:], hh[:],
                                             start=(j == 0), stop=(j == NT - 1))
                        # hsq rows: hi fp8 + (residual x16) fp8
                        nc.scalar.copy(hsqrow[0:1, 0:BS], hsqp[:])
                        hd = rowp.tile([1, BS], F32, tag="hd")
                        nc.vector.tensor_tensor(out=hd[:], in0=hsqp[:],
                                                in1=hsqrow[0:1, 0:BS],
                                                op=Alu.subtract)
                        nc.vector.tensor_scalar(
                            out=hsqrow[0:1, BS:2 * BS], in0=hd[:],
                            scalar1=16.0, scalar2=None, op0=Alu.mult)
                    else:
                        for j in range(NT):
                            js = slice(j * BS, (j + 1) * BS)
                            ot = scr.tile([128, BS], F32, tag="ot", bufs=3)
                            e0, e1 = ((nc.vector, nc.gpsimd),
                                      (nc.gpsimd, nc.vector))[j % 2]
                            e0.tensor_scalar(
                                out=ot[:], in0=e_all[:, js],
                                scalar1=s_t[:, j:j + 1], scalar2=u_t[:, j:j + 1],
                                op0=Alu.mult, op1=Alu.add)
                            e1.tensor_tensor(out=ot[:], in0=ot[:],
                                             in1=xres[:, js], op=Alu.add)
                            nc.sync.dma_start(outT[j * 128:(j + 1) * 128, :], ot[:])

            if loop_reps:
                with tc.For_i(0, loop_reps, 1):
                    body()
            else:
                for _rep in range(reps):
                    body()

    nc.compile()
    return nc


def _f8(a):
    return np.asarray(a, np.float32).astype(F8)


def _host_prep(x, mu1, lam1, v1, g1, b1, mu2, lam2, v2, g2, b2):
    """Build the device-input arrays (fp8 weights/activations, f32 consts)."""
    def chunkify(blk):
        # [D, 128] column block -> [128, CH*128] chunk-major partition layout
        return blk.reshape(CH, 128, 128).transpose(1, 0, 2).reshape(128, CH * 128)

    def pack_layer(mu, lam_, v, c_shift, x_shift):
        mu64 = mu.astype(np.float64)
        v64 = v.astype(np.float64)
        lam64 = lam_.astype(np.float64)
        Wmu = (-2.0 * mu64).T                              # [D, N] (no lam)
        vs = 16.0 * v64.transpose(1, 0, 2).reshape(K * N, D).T   # [D, K*N]
        v8 = _f8(vs)
        vr8 = _f8((vs - v8.astype(np.float64)) * 32.0)
        v832 = _f8(v8.astype(np.float32) / 32.0)
        blocks = []
        for j in range(NT):
            cols = slice(j * 128, (j + 1) * 128)
            blocks.append(_f8(chunkify(Wmu[:, cols])))
            for k in range(K):
                kc = slice(k * N + j * 128, k * N + (j + 1) * 128)
                blocks.append(chunkify(v8[:, kc]))
                blocks.append(chunkify(vr8[:, kc]))
                blocks.append(chunkify(v832[:, kc]))
        Wp = np.concatenate(blocks, axis=1)                # [128, WCOL] fp8
        # constants: proj = v.(x'+x_shift) - v.mu ; -2mu.(x'+x_shift)
        vm = (v64 * mu64[:, None, :]).sum(-1)              # [N, K]
        vsh = v64.sum(-1) * x_shift                        # [N, K]
        cv_l = (vsh - vm).reshape(NT, 128, K).transpose(1, 0, 2)
        musq = (mu64 * mu64).sum(1)
        mush = mu64.sum(1) * (2.0 * x_shift)
        ce = (-(lam64 * (musq + c_shift - mush)) / D).reshape(NT, 128).T
        lc = lam64.reshape(NT, 128).T
        return (Wp, cv_l.reshape(128, NT * K).astype(np.float32),
                ce.astype(np.float32), lc.astype(np.float32))

    W1pk, cv1, ce1, lc1 = pack_layer(mu1, lam1, v1, C1, 0.5)
    W2pk, cv2, ce2, lc2 = pack_layer(mu2, lam2, v2, 0.0, 0.0)
    cv_all = np.concatenate([cv1, cv2], axis=1)               # [128, 64]
    cexp = np.concatenate([ce1, ce2], axis=1)                 # [128, 16]
    lamc = np.concatenate([lc1, lc2], axis=1)                 # [128, 16]
    gbp = np.concatenate(
        [a.reshape(NT, 128).T for a in (g1, b1, g2, b2)],
        axis=1).astype(np.float32)                            # [128, 32]
    k1 = np.concatenate([np.ones(128), np.full(128, 1.0 / 16.0)])
    k1 = _f8(k1.reshape(1, 256))

    x64 = x.astype(np.float64)
    xT = np.ascontiguousarray(x.T)                            # [D, B]
    sqv = (x64 * x64).sum(1) - C1                             # [B]
    sq8 = _f8(sqv)
    sqlo = _f8((sqv - sq8.astype(np.float64)) * 16.0)

    in_maps = []
    for c in range(N_CORES):
        rs = slice(c * BS, (c + 1) * BS)
        xte = (xT[:, rs].reshape(CH, 128, BS).transpose(1, 0, 2)
               .reshape(128, CH * BS))
        x8 = _f8(xte - 0.5)
        xlo = _f8(x8.astype(np.float32) / 32.0)
        xr32 = _f8((xte - 0.5 - x8.astype(np.float64)) * 32.0)
        in_maps.append({
            "x8_s": x8, "xlo_s": xlo, "xr32_s": xr32,
            "xres_s": xte.astype(BF),
            "sqr_s": np.concatenate([sq8[rs], sqlo[rs]]).reshape(1, 2 * BS),
            "W1p": W1pk, "W2p": W2pk,
            "lamc_s": lamc, "cexp_s": cexp, "cv_s": cv_all, "gb_s": gbp,
            "k1_s": k1, "ones_s": np.ones((128, 1), F8),
        })
    return in_maps


def kernel(x, mu1, lam1, v1, g1, b1, mu2, lam2, v2, g2, b2):
    if "nc" not in _CACHE:
        _CACHE["nc"] = _build_nc()
    nc = _CACHE["nc"]
    in_maps = _host_prep(x, mu1, lam1, v1, g1, b1, mu2, lam2, v2, g2, b2)
    res = run_bass_kernel_spmd(nc, in_maps, list(range(N_CORES)))
    return np.concatenate(
        [res.results[c]["outT"].T for c in range(N_CORES)], axis=0)
